# revision 39
# baseline (speedup 1.0000x reference)
"""Non-local block (B=8, C=256, H=W=56) as a Bass/Tile kernel on 8 trn2 NeuronCores.

Sharding: pure data parallelism — core i computes sample i end-to-end
(attention is per-sample, weights replicated). kernel() takes the full
inputs, builds per-core input maps, runs the SPMD Bass program, and
stacks the per-core outputs.

Per-core math (xf = x[i] reshaped [C, N], N = H*W = 3136, CH = 128):
  theta = w_theta @ xf          [CH, N]   (bf16)
  phi   = w_phi   @ xf          [CH, N]   (bf16)
  gT    = (w_g @ xf)^T          [N, CH]   (fp8 e4m3, m-major)
  S_T[m, n] = (phi^T theta)[m, n]; E = exp(S_T - 3)        (fp8 e4m3)
  y[c, n] = sum_m gT[m, c] E[m, n]   (fp8 DoubleRow, PSUM-accumulated)
  d[n]    = sum_m E[m, n]            (ones-matmul, fp8 DoubleRow)
  out = w_z @ (y / d) + xf      [C, N]
The exp bias (-3) keeps E inside fp8 range and cancels exactly in y/d.

Structure: attention runs over 512-wide n-chunks x 13 m-units (12
DoubleRow pairs + 64-row tail). Each pair's two score blocks share one
[128, 1024] PSUM tile so a single ACT instruction exps the whole unit,
and its fp8 output [128, 2, 512] is exactly the DoubleRow rhs access
pattern. Scores run two units ahead of PV/d through 3 rotating PSUM
slots (lookahead-2) so the ScalarE exp pipe never starves — this is
worth ~15% end to end. Projections for the first unit are emitted up
front; the rest interleave into the first chunk's unit loop. Each
chunk's normalize/z/residual/store epilogue is deferred and emitted
after the next chunk's first scores so it overlaps. The final 64-wide
n-chunk packs 8 m-blocks of scores per PSUM bank (one exp per 8 blocks).
Stores are row-split 4-way across the sync/gpsimd/scalar queues (whole
1KB DMA packets, three parallel queues) so the final chunk's store
flight is short.

Tuning on top of that baseline (engine-busy from NTFF: PE ~92us, ACT
~85us, DVE ~40us over 121us):
  - exp offload: ACT (the only exp engine, 1 elem/lane/cycle) was
    saturated, so 5 of 13 units per steady chunk compute E on the DVE
    instead via a bit-trick: fp8e4m3 bits of exp(s-3) ~= trunc(A*s+B)
    (log-linear/Schraudolph; +-4% sawtooth, same noise family as the
    fp8 rounding itself, washes out in the 3136-term softmax average),
    as a SINGLE tensor_scalar writing through a uint8 bitcast: HW
    saturates f32->u8 (negatives -> 0, bit-identical to an explicit
    clamp; CoreSim wraps instead -- sim/HW divergence). gpsimd was
    tried for a clamp pass: 13us per instruction -- gpsimd is useless
    for elementwise work.
  - weights are transposed on the HOST (kernel() marshals w.T), killing
    the 8 PE transposes + identity + psum round-trips at startup.
  - chunk-0 x DMA descriptors fan across scalar/sync/gpsimd queues
    (DMA_DIRECT2D costs ~650ns serial per descriptor on one engine).
  - final chunk stores fan across 3 queues to shorten the tail flight.
  - theta/phi are fp8 (not bf16): same 1 cyc/row matmul rate, but FWL
    loads the per-m-block phi weights at 4B/cycle vs 2B, halving the
    scores LDWEIGHTS on the saturated PE queue.
  - HAM warmup: the PE boots throttled to K=4/8 (half clock) and only
    reaches K=8/8 after sustained matmul activity; the real matmuls
    can't start until x lands (~11us), so chunk 0 ran at half clock
    until ~26us. 14 dummy matmuls during the DMA-bound boot pull the
    un-throttle to ~10us. Filler matmuls in the tail chunk and final
    epilogue likewise bridge the late PE-idle gaps so the closing
    matmuls are not half-clocked by a re-armed throttle window.
    (Host-level drift of +-3.5us was observed on the shared device
    across this tuning session; the warmup/filler config was the most
    stable under it.)
Not worth it (measured): 1024-wide n-windows to amortize LDWEIGHTS
(PSUM only fits 2 score slots at that width -> PE stalls on the
slot ping-pong, 140us); InstMatmult.ldweights=False to dedupe the
constant all-ones d-weights (legalization emits LDWEIGHTS
unconditionally); uint8 DoubleColumn/DoublePixel 2x scores (bass
rejects uint8 matmul operands); lookahead-3 scores prefetch (PE
queue head then waits on a newer exp before PV can issue, +2us);
packing the tail chunk's y/d into a scores-pool PSUM tile (+3us);
wT-descriptors-first + x-casts-before-w-casts (x then lands later and
the whole chain shifts right); extending warmups to 24 and bridging
the post-projection DVE-copy hole with a third filler batch (both
displace or delay real work inside the 6.8us HAM probation window).
The HAM pattern is a FIXED cadence -- grant 6.8us, re-throttle 6.8us,
then the long grant -- so the second-window half-clock penalty during
early chunk 0 is unavoidable regardless of warmup placement (14/17/
20/22/24 warmups all measured; 14 is best).
Measured: ~119.2 us per core on trn2 (NTFF exec_time),
scale-relative absmax error ~1.2e-3 vs the fp32 reference.
"""

import os
import sys

import numpy as np

for _p in (
    "/opt/trn_rl_repo",
    "/root/.axon_site",
    "/root/.axon_site/_ro/trn_rl_repo",
    "/root/.axon_site/_ro/pypackages",
):
    if _p not in sys.path and os.path.isdir(_p):
        sys.path.append(_p)

import concourse.bass as bass  # noqa: E402
import concourse.bacc as bacc  # noqa: E402
import concourse.tile as tile  # noqa: E402
from concourse import mybir  # noqa: E402

B, C, H, W = 8, 256, 56, 56
N = H * W  # 3136
CH = C // 2  # 128
P = 128

CW = 512  # attention n-chunk width
# six 512-wide chunks + the 64-wide tail last (tiny final epilogue)
CHUNKS = [(i * CW, CW) for i in range(6)] + [(6 * CW, N - 6 * CW)]
MB = 25  # m blocks: 24 x 128 + 1 x 64
MB_TAIL = N - 24 * P  # 64
NPAIR = 12  # DoubleRow m-block pairs (0,1)...(22,23); mb 24 is the tail
DMACH = 784  # x DMA / cast chunk
PCH = 392  # projection free-dim chunk, 8 x 392 = 3136
EXP_BIAS = -3.0
EXP_SPL = 576  # ACT's share of a split pair-unit exp (of 1024 cols)

F32 = mybir.dt.float32
BF16 = mybir.dt.bfloat16
F8 = mybir.dt.float8e4

N_CORES = 8


def _mm_cols(width):
    """Split a free-dim width into <=512 column pieces (PSUM bank limit)."""
    cols, off = [], 0
    while off < width:
        w = min(512, width - off)
        cols.append((off, w))
        off += w
    return cols


def _kernel_body(tc):
    nc = tc.nc
    # weights arrive HOST-TRANSPOSED (see kernel()): the wT layouts feed the
    # PE lhsT directly, so no on-device PE transposes / identity are needed.
    x_d = nc.dram_tensor("x", [C, N], F32, kind="ExternalInput").ap()
    # all four weights packed on the HOST into one [128, 1024] f32 tensor
    # (4KB rows -> 4KB DMA packets; the separate per-weight transfers had
    # 512B packets and were starved by the x stream's 3136B packets):
    #   cols [j*384 + k*128 + m] = w_k[m, j*128+p]   k in (theta, phi, g)
    #   cols [768 + h*128 + c]   = w_z[h*128+c, p]
    wcat_d = nc.dram_tensor("w_cat", [P, 8 * P], F32, kind="ExternalInput").ap()
    out_d = nc.dram_tensor("out", [C, N], F32, kind="ExternalOutput").ap()

    from contextlib import ExitStack

    with ExitStack() as ctx:
        consts = ctx.enter_context(tc.tile_pool(name="consts", bufs=1))
        etp = ctx.enter_context(tc.tile_pool(name="etp", bufs=8))
        rp = ctx.enter_context(tc.tile_pool(name="rp", bufs=3))
        outp = ctx.enter_context(tc.tile_pool(name="outp", bufs=8))
        psum = ctx.enter_context(tc.tile_pool(name="psum", bufs=3, space="PSUM"))
        psum_y = ctx.enter_context(tc.tile_pool(name="psum_y", bufs=1, space="PSUM"))
        psum_d = ctx.enter_context(tc.tile_pool(name="psum_d", bufs=1, space="PSUM"))

        # ---- persistent SBUF tiles ----
        x_f32 = [consts.tile([P, N], F32, tag=f"x{h}", name=f"x{h}") for h in range(2)]
        # x in fp8, k-tile-major [c_half, h, n]: feeds DoubleRow projections
        # (contraction 256 = 2 k-tiles of 128) as both rhs (theta/phi) and
        # stationary (gT), halving projection PE passes vs bf16.
        x8 = consts.tile([P, 2, N], F8, tag="x8", name="x8")
        # theta/phi in fp8: the scores matmul runs at the same 1 cyc/row, but
        # FWL loads fp8 weights (phi blocks) at 4B/cycle vs 2B -> the per-
        # m-block LDWEIGHTS on the critical PE queue halves. The extra ~3%
        # operand noise washes out in the 3136-term softmax average.
        theta = consts.tile([P, N], F8, tag="theta", name="theta")
        phi = consts.tile([P, N], F8, tag="phi", name="phi")
        gT = consts.tile([P, MB * P], F8, tag="gT", name="gT")  # [m_local, mb*128+c]
        ynorm = consts.tile([P, N], BF16, tag="ynorm", name="ynorm")
        e_tail = consts.tile([P, CW], F8, tag="e_tail", name="e_tail")
        ones8 = consts.tile([P, 2, P], F8, tag="ones8", name="ones8")
        expb = consts.tile([P, 1], F32, tag="expb", name="expb")
        wcat_raw = consts.tile([P, 8 * P], F32, tag="wcat", name="wcat")
        # fp8 k-tile-major weights [c_half, h, th|ph|g] for the DR projections
        w8_all = consts.tile([P, 2, 3 * CH], F8, tag="w8", name="w8")
        KIDX = {"th": 0, "ph": 1, "g": 2}

        def w8k(k):
            return w8_all[:, :, KIDX[k] * CH : (KIDX[k] + 1) * CH]

        wzT = [
            consts.tile([P, P], BF16, tag=f"wzT{h}", name=f"wzT{h}") for h in range(2)
        ]

        # ---- load inputs. All DMA rings share ~334 GB/s aggregate with
        # ~round-robin arbitration, so the bulk x stream would starve the
        # startup-critical bytes (packed weights + x cols 0:784, ~1.3MB)
        # no matter how rings are assigned. Fix: the first BULK transfer on
        # each ring carries an explicit dependency on the critical set, so
        # rings 0-2 move only critical bytes until those land (~12us), and
        # the bulk still finishes at the same ~20us (bandwidth-bound).
        from concourse.bass import _add_dep_helper

        # Rings drain at ~equal per-ring byte rates and a ring executes its
        # descriptors IN ORDER — that in-ring ordering is the only reliable
        # prioritization (cross-ring dependency gates proved ineffective, and
        # the gpsimd software DGE ignores them entirely). So: ALL of x rides
        # the sync ring in need-order, the packed weights ride scalar alone,
        # and gpsimd carries nothing during the input phase.
        out_dma_engs = [nc.sync, nc.gpsimd]  # keep output DMAs off the ACT queue
        HF = DMACH // 2  # 392
        nc.scalar.dma_start(out=wcat_raw[:, : 6 * P], in_=wcat_d[:, : 6 * P])
        nc.scalar.dma_start(out=wcat_raw[:, 6 * P :], in_=wcat_d[:, 6 * P :])
        for h in range(2):
            nc.sync.dma_start(out=x_f32[h][:, 0:HF], in_=x_d[h * P : (h + 1) * P, 0:HF])
        for h in range(2):
            nc.sync.dma_start(
                out=x_f32[h][:, HF:DMACH], in_=x_d[h * P : (h + 1) * P, HF:DMACH]
            )
        for ci in range(1, N // DMACH):
            sl = slice(ci * DMACH, (ci + 1) * DMACH)
            for h in range(2):
                nc.sync.dma_start(
                    out=x_f32[h][:, sl], in_=x_d[h * P : (h + 1) * P, sl]
                )

        # ---- HAM warmup: the PE boots throttled to K=4/8 (half rate) and
        # only reaches K=8/8 after ~4us of sustained matmul activity. The
        # real matmuls can't start until x lands (~11us), so chunk 0 was
        # running at half clock until ~26us. Burn idle boot time on dummy
        # matmuls so the throttle lifts before the first projection.
        warm = consts.tile([P, CW], BF16, tag="warm", name="warm")
        nc.vector.memset(warm, 1.0)
        wu_ps = psum.tile([P, CW], F32, tag="s", name="wu")
        for _ in range(14):
            nc.tensor.matmul(wu_ps[:], warm[:, :P], warm[:], start=True, stop=True)

        nc.vector.memset(expb, EXP_BIAS)
        nc.vector.memset(ones8, 1.0)
        # only the padding regions of gT / e_tail need zeros; keep these off
        # the DVE queue (they'd delay the startup casts)
        nc.gpsimd.memset(gT[:, (MB - 1) * P :], 0.0)
        nc.gpsimd.memset(e_tail, 0.0)

        # ---- weight casts (weights pre-packed + transposed on the host) ----
        # on the ACT engine: it is idle until the first exp (~14us), while the
        # DVE's in-order queue must stay free for the x casts / proj copies
        nc.scalar.activation(
            out=w8_all.rearrange("p a b -> p (a b)"),
            in_=wcat_raw[:, : 6 * P],
            func=mybir.ActivationFunctionType.Copy,
        )

        # ---- deferred x casts + projection emitters (interleaved into
        # attention). Everything stays on DVE: the ACT queue must hold only
        # exps once the attention pipeline starts (in-order queue — a cast
        # waiting on a late DMA would block every exp behind it).
        xc_done = [0]

        # 392-wide casts: chunk 0's DMA lands as 392-wide subs, so the first
        # projection can start after the first sub instead of the full 784
        XCH = PCH  # 392

        DR = mybir.MatmulPerfMode.DoubleRow

        def ensure_xcast2(upto):
            upto = min(N // XCH, upto)
            while xc_done[0] < upto:
                j = xc_done[0]
                sl = slice(j * XCH, (j + 1) * XCH)
                for h in range(2):
                    nc.vector.tensor_copy(out=x8[:, h, sl], in_=x_f32[h][:, sl])
                xc_done[0] = j + 1

        def emit_proj_chunk(wkey, dst, j):
            ensure_xcast2(j + 1)
            sl = slice(j * PCH, (j + 1) * PCH)
            ps = psum.tile([P, PCH], F32, tag="s", name="s")
            # single fp8 DoubleRow pass: contraction 256 = 2 k-tiles of 128
            nc.tensor.matmul(
                ps[:], w8k(wkey), x8[:, :, sl], start=True, stop=True,
                perf_mode=DR,
            )
            nc.vector.tensor_copy(out=dst[:, sl], in_=ps[:])

        def emit_gt_block(mb):
            mw = P if mb < MB - 1 else MB_TAIL
            ensure_xcast2((mb * P + mw - 1) // XCH + 1)
            msl = slice(mb * P, mb * P + mw)
            ps = psum.tile([P, P], F32, tag="s", name="s")
            nc.tensor.matmul(
                ps[:mw, :], x8[:, :, msl], w8k("g"), start=True, stop=True,
                perf_mode=DR,
            )
            nc.vector.tensor_copy(out=gT[:mw, mb * P : (mb + 1) * P], in_=ps[:mw, :])

        done = {"th": 0, "ph": 0, "gT": 0}  # chunks/blocks emitted so far

        def ensure(kind, upto):
            """Emit projection work up to (exclusive) index `upto`."""
            while done[kind] < upto:
                j = done[kind]
                if kind == "th":
                    emit_proj_chunk("th", theta, j)
                elif kind == "ph":
                    emit_proj_chunk("ph", phi, j)
                else:
                    emit_gt_block(j)
                done[kind] = j + 1

        NP_CH = N // PCH  # 8

        def phi_chunks_for_cols(cols):
            return min(NP_CH, (cols + PCH - 1) // PCH)

        def theta_chunks_for_cols(cols):
            return min(NP_CH, (cols + PCH - 1) // PCH)

        # upfront: enough for unit 0 of chunk 0
        ensure("th", theta_chunks_for_cols(CW))  # theta cols 0:784 -> chunks 0,1
        ensure("ph", 1)  # phi cols 0:256
        ensure("gT", 2)  # m-blocks 0,1
        # wz casts on ACT too (idle until the first exp)
        for h in range(2):
            nc.scalar.activation(
                out=wzT[h][:],
                in_=wcat_raw[:, (6 + h) * P : (7 + h) * P],
                func=mybir.ActivationFunctionType.Copy,
            )

        # ---- attention ----
        DR = mybir.MatmulPerfMode.DoubleRow
        I8 = mybir.dt.int8
        # DVE bit-trick exp: fp8e4m3 bits of exp(s-3) ~= trunc(A*s + B)
        # (log-linear mantissa interpolation; error +-4% sawtooth, same
        # family as the fp8 rounding noise, washes out in the softmax avg).
        # B = 56 - 24*log2(e) + 0.156 (trunc-centering + Schraudolph shift).
        SCH_A = 11.541560327111707
        SCH_B = 21.5313

        def emit_dve_exp(dst_i8, src_ps):
            """dst = fp8 bits of exp(src-3) via affine+trunc on DVE; negatives
            (underflow) must clamp to 0 -- testing HW uint8 saturation."""
            nc.vector.tensor_scalar(
                out=dst_i8.bitcast(mybir.dt.uint8),
                in0=src_ps,
                scalar1=SCH_A,
                scalar2=SCH_B,
                op0=mybir.AluOpType.mult,
                op1=mybir.AluOpType.add,
            )

        def emit_scores_exp(cs, w, unit, interleave, use_dve=False, s_tile=None):
            """Scores+exp for all m-blocks of this unit; returns the E tile.

            For pair units both m-blocks' scores land in ONE [P, 2*CW] psum
            tile so a single ACT instruction exps the whole unit, and the
            fp8 output layout [P, 2, CW] is exactly the DoubleRow rhs AP."""
            if unit < NPAIR:
                et = etp.tile([P, 2, CW], F8, tag="et", name="et")
                mbs = (2 * unit, 2 * unit + 1)
            else:
                et = e_tail
                mbs = (MB - 1,)
            if interleave:
                # pre-requisites for this unit's scores and the NEXT unit's PV
                ensure("ph", phi_chunks_for_cols((mbs[-1] + 1) * P))
                if unit < NPAIR:
                    ensure("gT", min(MB, 2 * unit + 2))
                else:
                    ensure("gT", MB)
                # drain remaining theta early (needed from chunk 1 onward)
                ensure("th", min(NP_CH, theta_chunks_for_cols(CW) + unit))
            if unit < NPAIR:
                s_ps = psum.tile([P, 2 * CW], F32, tag="s", name="s")
                for j, mb in enumerate(mbs):
                    nc.tensor.matmul(
                        s_ps[:, j * CW : j * CW + w],
                        phi[:, mb * P : (mb + 1) * P],
                        theta[:, cs : cs + w],
                        start=True,
                        stop=True,
                    )
                if w == CW:
                    flat = et.rearrange("p a b -> p (a b)")
                    if use_dve == "split":
                        # split the unit's exp per k-tile across BOTH engines:
                        # the ACT half gates on only the FIRST scores matmul
                        # of the pair, so E is ready ~0.7us after the second
                        # matmul instead of ~1.15us — with lookahead-2 the PE
                        # has only ~0.86us of cover, so single-engine exp
                        # latency set the steady-state cadence
                        nc.scalar.activation(
                            out=et[:, 0, :],
                            in_=s_ps[:, :CW],
                            func=mybir.ActivationFunctionType.Exp,
                            bias=expb[:],
                        )
                        emit_dve_exp(et[:, 1, :].bitcast(I8), s_ps[:, CW:])
                    elif use_dve:
                        emit_dve_exp(flat.bitcast(I8), s_ps[:])
                    else:
                        nc.scalar.activation(
                            out=flat,
                            in_=s_ps[:],
                            func=mybir.ActivationFunctionType.Exp,
                            bias=expb[:],
                        )
                else:
                    for j in range(2):
                        nc.scalar.activation(
                            out=et[:, j, :w],
                            in_=s_ps[:, j * CW : j * CW + w],
                            func=mybir.ActivationFunctionType.Exp,
                            bias=expb[:],
                        )
            else:
                mb = MB - 1
                # when the tail unit leads the chunk (tail_first), its scores
                # borrow the d_ps bank (idle until PV starts) so the chunk's
                # opening scores don't contend for the 3 rotating slots
                s_ps = (
                    s_tile
                    if s_tile is not None
                    else psum.tile([P, 2 * CW], F32, tag="s", name="s")
                )
                nc.tensor.matmul(
                    s_ps[:MB_TAIL, :w],
                    phi[:, mb * P : mb * P + MB_TAIL],
                    theta[:, cs : cs + w],
                    start=True,
                    stop=True,
                )
                if use_dve:
                    emit_dve_exp(
                        et[:MB_TAIL, :w].bitcast(I8), s_ps[:MB_TAIL, :w]
                    )
                else:
                    nc.scalar.activation(
                        out=et[:MB_TAIL, :w],
                        in_=s_ps[:MB_TAIL, :w],
                        func=mybir.ActivationFunctionType.Exp,
                        bias=expb[:MB_TAIL],
                    )
            return et

        def emit_pv_d(y_ps, d_ps, et, w, unit, first, last):
            if unit < NPAIR:
                gpair = gT[:, 2 * unit * P : (2 * unit + 2) * P].rearrange(
                    "p (k c) -> p k c", k=2
                )
                for off, wdt in _mm_cols(w):
                    nc.tensor.matmul(
                        y_ps[:, off : off + wdt],
                        gpair,
                        et[:, :, off : off + wdt],
                        start=first,
                        stop=last,
                        perf_mode=DR,
                    )
                    nc.tensor.matmul(
                        d_ps[:, off : off + wdt],
                        ones8[:],
                        et[:, :, off : off + wdt],
                        start=first,
                        stop=last,
                        perf_mode=DR,
                    )
            else:
                for off, wdt in _mm_cols(w):
                    nc.tensor.matmul(
                        y_ps[:, off : off + wdt],
                        gT[:, (MB - 1) * P : MB * P],
                        et[:, off : off + wdt],
                        start=first,
                        stop=last,
                    )
                    nc.tensor.matmul(
                        d_ps[:, off : off + wdt],
                        ones8[:, 0, :],
                        et[:, off : off + wdt],
                        start=first,
                        stop=last,
                    )

        def wide_chunk(cs, w, interleave, flush, dve_mode=lambda u: False):
            y_ps = psum_y.tile([P, CW], F32, tag="y", name="y")
            d_ps = psum_d.tile([P, CW], F32, tag="d", name="d")
            order = list(range(NPAIR)) + [NPAIR]
            et_q = [emit_scores_exp(cs, w, order[0], interleave, dve_mode(0))]
            et_q.append(emit_scores_exp(cs, w, order[1], interleave, dve_mode(1)))
            for idx, unit in enumerate(order):
                first, last = idx == 0, idx == len(order) - 1
                if idx + 2 < len(order):
                    u2 = order[idx + 2]
                    et_q.append(
                        emit_scores_exp(cs, w, u2, interleave, dve_mode(u2))
                    )
                if idx == 0:
                    # previous chunk's epilogue: flushed after THREE units of
                    # scores so its z matmuls (gated on the DVE recip/mul
                    # chain) have ~650ns of PE cover at the chunk boundary
                    flush()
                emit_pv_d(y_ps, d_ps, et_q.pop(0), w, unit, first, last)
            return y_ps, d_ps

        def tail_chunk(cs, w, flush):
            """64-wide n-chunk: pack 8 m-blocks of scores per PSUM bank so a
            single ACT instruction covers 8 exps.

            """

            def group_scores_exp(g):
                if g < 3:
                    s8 = psum.tile([P, 8 * w], F32, tag="s", name="s8")
                    et8 = etp.tile([P, 8, w], F8, tag="et", name="et8")
                    for j in range(8):
                        mb = 8 * g + j
                        nc.tensor.matmul(
                            s8[:, j * w : (j + 1) * w],
                            phi[:, mb * P : (mb + 1) * P],
                            theta[:, cs : cs + w],
                            start=True,
                            stop=True,
                        )
                    nc.scalar.activation(
                        out=et8.rearrange("p a b -> p (a b)"),
                        in_=s8[:, : 8 * w],
                        func=mybir.ActivationFunctionType.Exp,
                        bias=expb[:],
                    )
                    return et8
                mb = MB - 1
                s_ps = psum.tile([P, CW], F32, tag="s", name="s")
                nc.tensor.matmul(
                    s_ps[:MB_TAIL, :w],
                    phi[:, mb * P : mb * P + MB_TAIL],
                    theta[:, cs : cs + w],
                    start=True,
                    stop=True,
                )
                nc.scalar.activation(
                    out=e_tail[:MB_TAIL, :w],
                    in_=s_ps[:MB_TAIL, :w],
                    func=mybir.ActivationFunctionType.Exp,
                    bias=expb[:MB_TAIL],
                )
                return e_tail

            y_ps = psum_y.tile([P, CW], F32, tag="y", name="y")
            d_ps = psum_d.tile([P, CW], F32, tag="d", name="d")
            et_q = [group_scores_exp(0)]
            flush()  # previous chunk's epilogue overlaps this chunk's scores
            # filler matmuls: keep the PE busy through the exp wait so the
            # HAM throttle doesn't re-arm for the final stretch
            fil = psum.tile([P, CW], F32, tag="s", name="fil")
            for _ in range(4):
                nc.tensor.matmul(fil[:], warm[:, :P], warm[:], start=True, stop=True)
            for g in range(4):
                if g + 1 <= 3:
                    et_q.append(group_scores_exp(g + 1))
                et_cur = et_q.pop(0)
                if g < 3:
                    for p_i in range(4):
                        pair = 4 * g + p_i
                        gpair = gT[
                            :, 2 * pair * P : (2 * pair + 2) * P
                        ].rearrange("p (k c) -> p k c", k=2)
                        first = pair == 0
                        nc.tensor.matmul(
                            y_ps[:, :w],
                            gpair,
                            et_cur[:, 2 * p_i : 2 * p_i + 2, :],
                            start=first,
                            stop=False,
                            perf_mode=DR,
                        )
                        nc.tensor.matmul(
                            d_ps[:, :w],
                            ones8[:],
                            et_cur[:, 2 * p_i : 2 * p_i + 2, :],
                            start=first,
                            stop=False,
                            perf_mode=DR,
                        )
                else:
                    nc.tensor.matmul(
                        y_ps[:, :w],
                        gT[:, (MB - 1) * P : MB * P],
                        et_cur[:, :w],
                        start=False,
                        stop=True,
                    )
                    nc.tensor.matmul(
                        d_ps[:, :w],
                        ones8[:, 0, :],
                        et_cur[:, :w],
                        start=False,
                        stop=True,
                    )
            return y_ps, d_ps

        def make_epilogue(ci, cs, w, y_ps, d_ps):
            def _ep():
                # normalize: ynorm = y / d  (d replicated across partitions)
                r_t = rp.tile([P, CW], F32, tag="r", name="r")
                nc.vector.reciprocal_approx_fast(
                    out=r_t[:, :w], in_=d_ps[:, :w]
                )
                nc.vector.tensor_mul(
                    out=ynorm[:, cs : cs + w], in0=y_ps[:, :w], in1=r_t[:, :w]
                )
                # z = w_z @ ynorm + x -> out. h=0 reuses the d slot (freed
                # after the recip), h=1 the y slot.
                if ci == len(CHUNKS) - 1:
                    # final epilogue: fill the PE's recip/mul wait with
                    # dummies so the closing z matmuls run at full clock
                    fil2 = psum.tile([P, CW], F32, tag="s", name="fil2")
                    for _ in range(10):
                        nc.tensor.matmul(
                            fil2[:], warm[:, :P], warm[:], start=True, stop=True
                        )
                for h, ztag in ((0, "d"), (1, "y")):
                    zpool = psum_y if ztag == "y" else psum_d
                    z_ps = zpool.tile([P, CW], F32, tag=ztag, name="zps")
                    nc.tensor.matmul(
                        z_ps[:, :w],
                        wzT[h][:],
                        ynorm[:, cs : cs + w],
                        start=True,
                        stop=True,
                    )
                    o_t = outp.tile([P, CW], F32, tag="o", name="o")
                    nc.vector.tensor_add(
                        out=o_t[:, :w], in0=z_ps[:, :w], in1=x_f32[h][:, cs : cs + w]
                    )
                    half = (w + 1) // 2
                    final = ci == len(CHUNKS) - 1
                    for s, (so, sw) in enumerate(((0, half), (half, w - half))):
                        if sw <= 0:
                            continue
                        if final:
                            # last chunk: fan the stores over 3 queues so the
                            # closing store flight is as short as possible
                            eng = (nc.sync, nc.gpsimd, nc.scalar, nc.sync)[
                                2 * h + s
                            ]
                        else:
                            eng = out_dma_engs[(h + ci + s) % 2]
                        eng.dma_start(
                            out=out_d[h * P : (h + 1) * P, cs + so : cs + so + sw],
                            in_=o_t[:, so : so + sw],
                        )

            return _ep

        pending = [None]

        def flush():
            if pending[0] is not None:
                pending[0]()
                pending[0] = None

        # Steady chunks split every pair-unit's exp across ACT+DVE (see
        # emit_scores_exp). Chunk 0's DVE is busy with interleaved
        # projections -> ACT-only there; the tail m-unit stays on DVE.
        def steady_units(u):
            return "split" if u < NPAIR else False  # tail unit's exp on ACT

        for ci, (cs, w) in enumerate(CHUNKS):
            if w > 64:
                y_ps, d_ps = wide_chunk(
                    cs,
                    w,
                    interleave=(ci == 0),
                    flush=flush,
                    dve_mode=(lambda u: False) if ci == 0 else steady_units,
                )
            else:
                y_ps, d_ps = tail_chunk(cs, w, flush=flush)
            pending[0] = make_epilogue(ci, cs, w, y_ps, d_ps)
        flush()

        assert done == {"th": NP_CH, "ph": NP_CH, "gT": MB}, done


_NC_CACHE = None


def build_nc():
    global _NC_CACHE
    if _NC_CACHE is None:
        nc = bacc.Bacc("TRN2", target_bir_lowering=False, debug=False)
        with tile.TileContext(nc) as tc:
            _kernel_body(tc)
        nc.compile()
        _NC_CACHE = nc
    return _NC_CACHE


def kernel(x, w_theta, w_phi, w_g, w_z, trace=False):
    assert x.shape == (B, C, H, W), x.shape
    nc = build_nc()
    from concourse.bass_utils import run_bass_kernel_spmd

    # pack all weights into one [128, 1024] f32 tensor (see _kernel_body):
    # [ th_j0 | ph_j0 | g_j0 | th_j1 | ph_j1 | g_j1 | wz.T ] with
    # w_k_jh[p, m] = w_k[m, h*128+p]
    wth_t = np.asarray(w_theta, dtype=np.float32).T  # [C, CH]
    wph_t = np.asarray(w_phi, dtype=np.float32).T
    wg_t = np.asarray(w_g, dtype=np.float32).T
    wz_t = np.asarray(w_z, dtype=np.float32).T  # [CH, C]
    blocks = []
    for h in range(2):
        for wt in (wth_t, wph_t, wg_t):
            blocks.append(wt[h * 128 : (h + 1) * 128, :])
    blocks.append(wz_t)
    shared = {
        "w_cat": np.ascontiguousarray(np.concatenate(blocks, axis=1), np.float32)
    }
    in_maps = [
        dict(shared, x=np.ascontiguousarray(x[i].reshape(C, N), dtype=np.float32))
        for i in range(N_CORES)
    ]
    res = run_bass_kernel_spmd(
        nc, in_maps, core_ids=list(range(N_CORES)), trace=trace
    )
    out = np.stack([res.results[i]["out"].reshape(C, H, W) for i in range(N_CORES)])
    kernel.last_result = res
    return out


kernel.last_result = None



# revision 40
# speedup vs baseline: 1.0385x; 1.0385x over previous
"""Non-local block (B=8, C=256, H=W=56) as a Bass/Tile kernel on 8 trn2 NeuronCores.

Sharding: pure data parallelism — core i computes sample i end-to-end
(attention is per-sample, weights replicated). kernel() takes the full
inputs, builds per-core input maps, runs the SPMD Bass program, and
stacks the per-core outputs.

Per-core math (xf = x[i] reshaped [C, N], N = H*W = 3136, CH = 128):
  theta = w_theta @ xf          [CH, N]   (bf16)
  phi   = w_phi   @ xf          [CH, N]   (bf16)
  gT    = (w_g @ xf)^T          [N, CH]   (fp8 e4m3, m-major)
  S_T[m, n] = (phi^T theta)[m, n]; E = exp(S_T - 3)        (fp8 e4m3)
  y[c, n] = sum_m gT[m, c] E[m, n]   (fp8 DoubleRow, PSUM-accumulated)
  d[n]    = sum_m E[m, n]            (ones-matmul, fp8 DoubleRow)
  out = w_z @ (y / d) + xf      [C, N]
The exp bias (-3) keeps E inside fp8 range and cancels exactly in y/d.

Structure: attention runs over 512-wide n-chunks x 13 m-units (12
DoubleRow pairs + 64-row tail). Each pair's two score blocks share one
[128, 1024] PSUM tile so a single ACT instruction exps the whole unit,
and its fp8 output [128, 2, 512] is exactly the DoubleRow rhs access
pattern. Scores run two units ahead of PV/d through 3 rotating PSUM
slots (lookahead-2) so the ScalarE exp pipe never starves — this is
worth ~15% end to end. Projections for the first unit are emitted up
front; the rest interleave into the first chunk's unit loop. Each
chunk's normalize/z/residual/store epilogue is deferred and emitted
after the next chunk's first scores so it overlaps. The final 64-wide
n-chunk packs 8 m-blocks of scores per PSUM bank (one exp per 8 blocks).
Stores are row-split 4-way across the sync/gpsimd/scalar queues (whole
1KB DMA packets, three parallel queues) so the final chunk's store
flight is short.

Tuning on top of that baseline (engine-busy from NTFF: PE ~92us, ACT
~85us, DVE ~40us over 121us):
  - exp offload: ACT (the only exp engine, 1 elem/lane/cycle) was
    saturated, so 5 of 13 units per steady chunk compute E on the DVE
    instead via a bit-trick: fp8e4m3 bits of exp(s-3) ~= trunc(A*s+B)
    (log-linear/Schraudolph; +-4% sawtooth, same noise family as the
    fp8 rounding itself, washes out in the 3136-term softmax average),
    as a SINGLE tensor_scalar writing through a uint8 bitcast: HW
    saturates f32->u8 (negatives -> 0, bit-identical to an explicit
    clamp; CoreSim wraps instead -- sim/HW divergence). gpsimd was
    tried for a clamp pass: 13us per instruction -- gpsimd is useless
    for elementwise work.
  - weights are transposed on the HOST (kernel() marshals w.T), killing
    the 8 PE transposes + identity + psum round-trips at startup.
  - chunk-0 x DMA descriptors fan across scalar/sync/gpsimd queues
    (DMA_DIRECT2D costs ~650ns serial per descriptor on one engine).
  - final chunk stores fan across 3 queues to shorten the tail flight.
  - theta/phi are fp8 (not bf16): same 1 cyc/row matmul rate, but FWL
    loads the per-m-block phi weights at 4B/cycle vs 2B, halving the
    scores LDWEIGHTS on the saturated PE queue.
  - HAM warmup: the PE boots throttled to K=4/8 (half clock) and only
    reaches K=8/8 after sustained matmul activity; the real matmuls
    can't start until x lands (~11us), so chunk 0 ran at half clock
    until ~26us. 14 dummy matmuls during the DMA-bound boot pull the
    un-throttle to ~10us. Filler matmuls in the tail chunk and final
    epilogue likewise bridge the late PE-idle gaps so the closing
    matmuls are not half-clocked by a re-armed throttle window.
    (Host-level drift of +-3.5us was observed on the shared device
    across this tuning session; the warmup/filler config was the most
    stable under it.)
Not worth it (measured): 1024-wide n-windows to amortize LDWEIGHTS
(PSUM only fits 2 score slots at that width -> PE stalls on the
slot ping-pong, 140us); InstMatmult.ldweights=False to dedupe the
constant all-ones d-weights (legalization emits LDWEIGHTS
unconditionally); uint8 DoubleColumn/DoublePixel 2x scores (bass
rejects uint8 matmul operands); lookahead-3 scores prefetch (PE
queue head then waits on a newer exp before PV can issue, +2us);
packing the tail chunk's y/d into a scores-pool PSUM tile (+3us);
wT-descriptors-first + x-casts-before-w-casts (x then lands later and
the whole chain shifts right); extending warmups to 24 and bridging
the post-projection DVE-copy hole with a third filler batch (both
displace or delay real work inside the 6.8us HAM probation window).
The HAM pattern is a FIXED cadence -- grant 6.8us, re-throttle 6.8us,
then the long grant -- so the second-window half-clock penalty during
early chunk 0 is unavoidable regardless of warmup placement (14/17/
20/22/24 warmups all measured; 14 is best).
Measured: ~119.2 us per core on trn2 (NTFF exec_time),
scale-relative absmax error ~1.2e-3 vs the fp32 reference.
"""

import os
import sys

import numpy as np

for _p in (
    "/opt/trn_rl_repo",
    "/root/.axon_site",
    "/root/.axon_site/_ro/trn_rl_repo",
    "/root/.axon_site/_ro/pypackages",
):
    if _p not in sys.path and os.path.isdir(_p):
        sys.path.append(_p)

import concourse.bass as bass  # noqa: E402
import concourse.bacc as bacc  # noqa: E402
import concourse.tile as tile  # noqa: E402
from concourse import mybir  # noqa: E402

B, C, H, W = 8, 256, 56, 56
N = H * W  # 3136
CH = C // 2  # 128
P = 128

CW = 512  # attention n-chunk width
# six 512-wide chunks + the 64-wide tail last (tiny final epilogue)
CHUNKS = [(i * CW, CW) for i in range(6)] + [(6 * CW, N - 6 * CW)]
MB = 25  # m blocks: 24 x 128 + 1 x 64
MB_TAIL = N - 24 * P  # 64
NPAIR = 12  # DoubleRow m-block pairs (0,1)...(22,23); mb 24 is the tail
DMACH = 784  # x DMA / cast chunk
PCH = 392  # projection free-dim chunk, 8 x 392 = 3136
EXP_BIAS = -3.0
EXP_SPL = 576  # ACT's share of a split pair-unit exp (of 1024 cols)

F32 = mybir.dt.float32
BF16 = mybir.dt.bfloat16
F8 = mybir.dt.float8e4

N_CORES = 8


def _mm_cols(width):
    """Split a free-dim width into <=512 column pieces (PSUM bank limit)."""
    cols, off = [], 0
    while off < width:
        w = min(512, width - off)
        cols.append((off, w))
        off += w
    return cols


def _kernel_body(tc):
    nc = tc.nc
    # weights arrive HOST-TRANSPOSED (see kernel()): the wT layouts feed the
    # PE lhsT directly, so no on-device PE transposes / identity are needed.
    x_d = nc.dram_tensor("x", [C, N], F32, kind="ExternalInput").ap()
    # all four weights packed on the HOST into one [128, 1024] f32 tensor
    # (4KB rows -> 4KB DMA packets; the separate per-weight transfers had
    # 512B packets and were starved by the x stream's 3136B packets):
    #   cols [j*384 + k*128 + m] = w_k[m, j*128+p]   k in (theta, phi, g)
    #   cols [768 + h*128 + c]   = w_z[h*128+c, p]
    wcat_d = nc.dram_tensor("w_cat", [P, 8 * P], F32, kind="ExternalInput").ap()
    out_d = nc.dram_tensor("out", [C, N], F32, kind="ExternalOutput").ap()

    from contextlib import ExitStack

    with ExitStack() as ctx:
        consts = ctx.enter_context(tc.tile_pool(name="consts", bufs=1))
        etp = ctx.enter_context(tc.tile_pool(name="etp", bufs=8))
        rp = ctx.enter_context(tc.tile_pool(name="rp", bufs=3))
        outp = ctx.enter_context(tc.tile_pool(name="outp", bufs=8))
        psum = ctx.enter_context(tc.tile_pool(name="psum", bufs=3, space="PSUM"))
        psum_y = ctx.enter_context(tc.tile_pool(name="psum_y", bufs=1, space="PSUM"))
        psum_d = ctx.enter_context(tc.tile_pool(name="psum_d", bufs=1, space="PSUM"))

        # ---- persistent SBUF tiles ----
        x_f32 = [consts.tile([P, N], F32, tag=f"x{h}", name=f"x{h}") for h in range(2)]
        # x in fp8, k-tile-major [c_half, h, n]: feeds DoubleRow projections
        # (contraction 256 = 2 k-tiles of 128) as both rhs (theta/phi) and
        # stationary (gT), halving projection PE passes vs bf16.
        x8 = consts.tile([P, 2, N], F8, tag="x8", name="x8")
        # theta/phi in fp8: the scores matmul runs at the same 1 cyc/row, but
        # FWL loads fp8 weights (phi blocks) at 4B/cycle vs 2B -> the per-
        # m-block LDWEIGHTS on the critical PE queue halves. The extra ~3%
        # operand noise washes out in the 3136-term softmax average.
        theta = consts.tile([P, N], F8, tag="theta", name="theta")
        phi = consts.tile([P, N], F8, tag="phi", name="phi")
        gT = consts.tile([P, MB * P], F8, tag="gT", name="gT")  # [m_local, mb*128+c]
        ynorm = consts.tile([P, N], BF16, tag="ynorm", name="ynorm")
        e_tail = consts.tile([P, CW], F8, tag="e_tail", name="e_tail")
        ones8 = consts.tile([P, 2, P], F8, tag="ones8", name="ones8")
        expb = consts.tile([P, 1], F32, tag="expb", name="expb")
        wcat_raw = consts.tile([P, 8 * P], F32, tag="wcat", name="wcat")
        # fp8 k-tile-major weights [c_half, h, th|ph|g] for the DR projections
        w8_all = consts.tile([P, 2, 3 * CH], F8, tag="w8", name="w8")
        KIDX = {"th": 0, "ph": 1, "g": 2}

        def w8k(k):
            return w8_all[:, :, KIDX[k] * CH : (KIDX[k] + 1) * CH]

        wzT = [
            consts.tile([P, P], BF16, tag=f"wzT{h}", name=f"wzT{h}") for h in range(2)
        ]

        # ---- load inputs. All DMA rings share ~334 GB/s aggregate with
        # ~round-robin arbitration, so the bulk x stream would starve the
        # startup-critical bytes (packed weights + x cols 0:784, ~1.3MB)
        # no matter how rings are assigned. Fix: the first BULK transfer on
        # each ring carries an explicit dependency on the critical set, so
        # rings 0-2 move only critical bytes until those land (~12us), and
        # the bulk still finishes at the same ~20us (bandwidth-bound).
        from concourse.bass import _add_dep_helper

        # Rings drain at ~equal per-ring byte rates and a ring executes its
        # descriptors IN ORDER — that in-ring ordering is the only reliable
        # prioritization (cross-ring dependency gates proved ineffective, and
        # the gpsimd software DGE ignores them entirely). So: ALL of x rides
        # the sync ring in need-order, the packed weights ride scalar alone,
        # and gpsimd carries nothing during the input phase.
        out_dma_engs = [nc.sync, nc.gpsimd]  # keep output DMAs off the ACT queue
        HF = DMACH // 2  # 392
        nc.scalar.dma_start(out=wcat_raw[:, : 6 * P], in_=wcat_d[:, : 6 * P])
        nc.scalar.dma_start(out=wcat_raw[:, 6 * P :], in_=wcat_d[:, 6 * P :])
        for h in range(2):
            nc.sync.dma_start(out=x_f32[h][:, 0:HF], in_=x_d[h * P : (h + 1) * P, 0:HF])
        for h in range(2):
            nc.sync.dma_start(
                out=x_f32[h][:, HF:DMACH], in_=x_d[h * P : (h + 1) * P, HF:DMACH]
            )
        for ci in range(1, N // DMACH):
            sl = slice(ci * DMACH, (ci + 1) * DMACH)
            for h in range(2):
                nc.sync.dma_start(
                    out=x_f32[h][:, sl], in_=x_d[h * P : (h + 1) * P, sl]
                )

        # ---- HAM warmup: the PE boots throttled to K=4/8 (half rate) and
        # only reaches K=8/8 after ~4us of sustained matmul activity. The
        # real matmuls can't start until x lands (~11us), so chunk 0 was
        # running at half clock until ~26us. Burn idle boot time on dummy
        # matmuls so the throttle lifts before the first projection.
        warm = consts.tile([P, CW], BF16, tag="warm", name="warm")
        nc.vector.memset(warm, 1.0)
        wu_ps = psum.tile([P, CW], F32, tag="s", name="wu")
        for _ in range(14):
            nc.tensor.matmul(wu_ps[:], warm[:, :P], warm[:], start=True, stop=True)

        nc.vector.memset(expb, EXP_BIAS)
        nc.vector.memset(ones8, 1.0)
        # only the padding regions of gT / e_tail need zeros; keep these off
        # the DVE queue (they'd delay the startup casts)
        nc.gpsimd.memset(gT[:, (MB - 1) * P :], 0.0)
        nc.gpsimd.memset(e_tail, 0.0)

        # ---- weight casts (weights pre-packed + transposed on the host) ----
        # on the ACT engine: it is idle until the first exp (~14us), while the
        # DVE's in-order queue must stay free for the x casts / proj copies
        nc.scalar.activation(
            out=w8_all.rearrange("p a b -> p (a b)"),
            in_=wcat_raw[:, : 6 * P],
            func=mybir.ActivationFunctionType.Copy,
        )

        # ---- deferred x casts + projection emitters (interleaved into
        # attention). Everything stays on DVE: the ACT queue must hold only
        # exps once the attention pipeline starts (in-order queue — a cast
        # waiting on a late DMA would block every exp behind it).
        xc_done = [0]

        # 392-wide casts: chunk 0's DMA lands as 392-wide subs, so the first
        # projection can start after the first sub instead of the full 784
        XCH = PCH  # 392

        DR = mybir.MatmulPerfMode.DoubleRow

        def ensure_xcast2(upto):
            upto = min(N // XCH, upto)
            while xc_done[0] < upto:
                j = xc_done[0]
                sl = slice(j * XCH, (j + 1) * XCH)
                for h in range(2):
                    nc.vector.tensor_copy(out=x8[:, h, sl], in_=x_f32[h][:, sl])
                xc_done[0] = j + 1

        def emit_proj_chunk(wkey, dst, j):
            ensure_xcast2(j + 1)
            sl = slice(j * PCH, (j + 1) * PCH)
            ps = psum.tile([P, PCH], F32, tag="s", name="s")
            # single fp8 DoubleRow pass: contraction 256 = 2 k-tiles of 128
            nc.tensor.matmul(
                ps[:], w8k(wkey), x8[:, :, sl], start=True, stop=True,
                perf_mode=DR,
            )
            nc.vector.tensor_copy(out=dst[:, sl], in_=ps[:])

        def emit_gt_block(mb):
            mw = P if mb < MB - 1 else MB_TAIL
            ensure_xcast2((mb * P + mw - 1) // XCH + 1)
            msl = slice(mb * P, mb * P + mw)
            ps = psum.tile([P, P], F32, tag="s", name="s")
            nc.tensor.matmul(
                ps[:mw, :], x8[:, :, msl], w8k("g"), start=True, stop=True,
                perf_mode=DR,
            )
            nc.vector.tensor_copy(out=gT[:mw, mb * P : (mb + 1) * P], in_=ps[:mw, :])

        done = {"th": 0, "ph": 0, "gT": 0}  # chunks/blocks emitted so far

        def ensure(kind, upto):
            """Emit projection work up to (exclusive) index `upto`."""
            while done[kind] < upto:
                j = done[kind]
                if kind == "th":
                    emit_proj_chunk("th", theta, j)
                elif kind == "ph":
                    emit_proj_chunk("ph", phi, j)
                else:
                    emit_gt_block(j)
                done[kind] = j + 1

        NP_CH = N // PCH  # 8

        def phi_chunks_for_cols(cols):
            return min(NP_CH, (cols + PCH - 1) // PCH)

        def theta_chunks_for_cols(cols):
            return min(NP_CH, (cols + PCH - 1) // PCH)

        # upfront: enough for unit 0 of chunk 0
        ensure("th", theta_chunks_for_cols(CW))  # theta cols 0:784 -> chunks 0,1
        ensure("ph", 1)  # phi cols 0:256
        ensure("gT", 2)  # m-blocks 0,1
        # wz casts on ACT too (idle until the first exp)
        for h in range(2):
            nc.scalar.activation(
                out=wzT[h][:],
                in_=wcat_raw[:, (6 + h) * P : (7 + h) * P],
                func=mybir.ActivationFunctionType.Copy,
            )

        # ---- attention ----
        DR = mybir.MatmulPerfMode.DoubleRow
        I8 = mybir.dt.int8
        # DVE bit-trick exp: fp8e4m3 bits of exp(s-3) ~= trunc(A*s + B)
        # (log-linear mantissa interpolation; error +-4% sawtooth, same
        # family as the fp8 rounding noise, washes out in the softmax avg).
        # B = 56 - 24*log2(e) + 0.156 (trunc-centering + Schraudolph shift).
        SCH_A = 11.541560327111707
        SCH_B = 21.5313

        def emit_dve_exp(dst_i8, src_ps):
            """dst = fp8 bits of exp(src-3) via affine+trunc on DVE; negatives
            (underflow) must clamp to 0 -- testing HW uint8 saturation."""
            nc.vector.tensor_scalar(
                out=dst_i8.bitcast(mybir.dt.uint8),
                in0=src_ps,
                scalar1=SCH_A,
                scalar2=SCH_B,
                op0=mybir.AluOpType.mult,
                op1=mybir.AluOpType.add,
            )

        def emit_scores_exp(cs, w, unit, interleave, use_dve=False, s_tile=None):
            """Scores+exp for all m-blocks of this unit; returns the E tile.

            For pair units both m-blocks' scores land in ONE [P, 2*CW] psum
            tile so a single ACT instruction exps the whole unit, and the
            fp8 output layout [P, 2, CW] is exactly the DoubleRow rhs AP."""
            if unit < NPAIR:
                et = etp.tile([P, 2, CW], F8, tag="et", name="et")
                mbs = (2 * unit, 2 * unit + 1)
            else:
                et = e_tail
                mbs = (MB - 1,)
            if interleave:
                # pre-requisites for this unit's scores and the NEXT unit's PV
                ensure("ph", phi_chunks_for_cols((mbs[-1] + 1) * P))
                if unit < NPAIR:
                    ensure("gT", min(MB, 2 * unit + 2))
                else:
                    ensure("gT", MB)
                # drain remaining theta early (needed from chunk 1 onward)
                ensure("th", min(NP_CH, theta_chunks_for_cols(CW) + unit))
            if unit < NPAIR:
                s_ps = psum.tile([P, 2 * CW], F32, tag="s", name="s")
                for j, mb in enumerate(mbs):
                    nc.tensor.matmul(
                        s_ps[:, j * CW : j * CW + w],
                        phi[:, mb * P : (mb + 1) * P],
                        theta[:, cs : cs + w],
                        start=True,
                        stop=True,
                    )
                if w == CW:
                    flat = et.rearrange("p a b -> p (a b)")
                    if use_dve == "split":
                        # split the unit's exp per k-tile across BOTH engines:
                        # the ACT half gates on only the FIRST scores matmul
                        # of the pair, so E is ready ~0.7us after the second
                        # matmul instead of ~1.15us — with lookahead-2 the PE
                        # has only ~0.86us of cover, so single-engine exp
                        # latency set the steady-state cadence
                        nc.scalar.activation(
                            out=et[:, 0, :],
                            in_=s_ps[:, :CW],
                            func=mybir.ActivationFunctionType.Exp,
                            bias=expb[:],
                        )
                        emit_dve_exp(et[:, 1, :].bitcast(I8), s_ps[:, CW:])
                    elif use_dve:
                        emit_dve_exp(flat.bitcast(I8), s_ps[:])
                    else:
                        nc.scalar.activation(
                            out=flat,
                            in_=s_ps[:],
                            func=mybir.ActivationFunctionType.Exp,
                            bias=expb[:],
                        )
                else:
                    for j in range(2):
                        nc.scalar.activation(
                            out=et[:, j, :w],
                            in_=s_ps[:, j * CW : j * CW + w],
                            func=mybir.ActivationFunctionType.Exp,
                            bias=expb[:],
                        )
            else:
                mb = MB - 1
                # when the tail unit leads the chunk (tail_first), its scores
                # borrow the d_ps bank (idle until PV starts) so the chunk's
                # opening scores don't contend for the 3 rotating slots
                s_ps = (
                    s_tile
                    if s_tile is not None
                    else psum.tile([P, 2 * CW], F32, tag="s", name="s")
                )
                nc.tensor.matmul(
                    s_ps[:MB_TAIL, :w],
                    phi[:, mb * P : mb * P + MB_TAIL],
                    theta[:, cs : cs + w],
                    start=True,
                    stop=True,
                )
                if use_dve:
                    emit_dve_exp(
                        et[:MB_TAIL, :w].bitcast(I8), s_ps[:MB_TAIL, :w]
                    )
                else:
                    nc.scalar.activation(
                        out=et[:MB_TAIL, :w],
                        in_=s_ps[:MB_TAIL, :w],
                        func=mybir.ActivationFunctionType.Exp,
                        bias=expb[:MB_TAIL],
                    )
            return et

        def emit_pv_d(y_ps, d_ps, et, w, unit, first, last):
            if unit < NPAIR:
                gpair = gT[:, 2 * unit * P : (2 * unit + 2) * P].rearrange(
                    "p (k c) -> p k c", k=2
                )
                for off, wdt in _mm_cols(w):
                    nc.tensor.matmul(
                        y_ps[:, off : off + wdt],
                        gpair,
                        et[:, :, off : off + wdt],
                        start=first,
                        stop=last,
                        perf_mode=DR,
                    )
                    nc.tensor.matmul(
                        d_ps[:, off : off + wdt],
                        ones8[:],
                        et[:, :, off : off + wdt],
                        start=first,
                        stop=last,
                        perf_mode=DR,
                    )
            else:
                for off, wdt in _mm_cols(w):
                    nc.tensor.matmul(
                        y_ps[:, off : off + wdt],
                        gT[:, (MB - 1) * P : MB * P],
                        et[:, off : off + wdt],
                        start=first,
                        stop=last,
                    )
                    nc.tensor.matmul(
                        d_ps[:, off : off + wdt],
                        ones8[:, 0, :],
                        et[:, off : off + wdt],
                        start=first,
                        stop=last,
                    )

        def wide_chunk(cs, w, interleave, flush, dve_mode=lambda u: False):
            y_ps = psum_y.tile([P, CW], F32, tag="y", name="y")
            d_ps = psum_d.tile([P, CW], F32, tag="d", name="d")
            order = list(range(NPAIR)) + [NPAIR]
            et_q = [emit_scores_exp(cs, w, order[0], interleave, dve_mode(0))]
            et_q.append(emit_scores_exp(cs, w, order[1], interleave, dve_mode(1)))
            for idx, unit in enumerate(order):
                first, last = idx == 0, idx == len(order) - 1
                if idx + 2 < len(order):
                    u2 = order[idx + 2]
                    et_q.append(
                        emit_scores_exp(cs, w, u2, interleave, dve_mode(u2))
                    )
                if idx == 0:
                    # previous chunk's epilogue: flushed after THREE units of
                    # scores so its z matmuls (gated on the DVE recip/mul
                    # chain) have ~650ns of PE cover at the chunk boundary
                    flush()
                emit_pv_d(y_ps, d_ps, et_q.pop(0), w, unit, first, last)
            return y_ps, d_ps

        def tail_chunk(cs, w, flush):
            """64-wide n-chunk: pack 8 m-blocks of scores per PSUM bank so a
            single ACT instruction covers 8 exps.

            """

            def group_scores_exp(g):
                if g < 3:
                    s8 = psum.tile([P, 8 * w], F32, tag="s", name="s8")
                    et8 = etp.tile([P, 8, w], F8, tag="et", name="et8")
                    for j in range(8):
                        mb = 8 * g + j
                        nc.tensor.matmul(
                            s8[:, j * w : (j + 1) * w],
                            phi[:, mb * P : (mb + 1) * P],
                            theta[:, cs : cs + w],
                            start=True,
                            stop=True,
                        )
                    nc.scalar.activation(
                        out=et8.rearrange("p a b -> p (a b)"),
                        in_=s8[:, : 8 * w],
                        func=mybir.ActivationFunctionType.Exp,
                        bias=expb[:],
                    )
                    return et8
                mb = MB - 1
                s_ps = psum.tile([P, CW], F32, tag="s", name="s")
                nc.tensor.matmul(
                    s_ps[:MB_TAIL, :w],
                    phi[:, mb * P : mb * P + MB_TAIL],
                    theta[:, cs : cs + w],
                    start=True,
                    stop=True,
                )
                nc.scalar.activation(
                    out=e_tail[:MB_TAIL, :w],
                    in_=s_ps[:MB_TAIL, :w],
                    func=mybir.ActivationFunctionType.Exp,
                    bias=expb[:MB_TAIL],
                )
                return e_tail

            y_ps = psum_y.tile([P, CW], F32, tag="y", name="y")
            d_ps = psum_d.tile([P, CW], F32, tag="d", name="d")
            et_q = [group_scores_exp(0)]
            flush()  # previous chunk's epilogue overlaps this chunk's scores
            # filler matmuls: keep the PE busy through the exp wait so the
            # HAM throttle doesn't re-arm for the final stretch
            fil = psum.tile([P, CW], F32, tag="s", name="fil")
            for _ in range(4):
                nc.tensor.matmul(fil[:], warm[:, :P], warm[:], start=True, stop=True)
            for g in range(4):
                if g + 1 <= 3:
                    et_q.append(group_scores_exp(g + 1))
                et_cur = et_q.pop(0)
                if g < 3:
                    for p_i in range(4):
                        pair = 4 * g + p_i
                        gpair = gT[
                            :, 2 * pair * P : (2 * pair + 2) * P
                        ].rearrange("p (k c) -> p k c", k=2)
                        first = pair == 0
                        nc.tensor.matmul(
                            y_ps[:, :w],
                            gpair,
                            et_cur[:, 2 * p_i : 2 * p_i + 2, :],
                            start=first,
                            stop=False,
                            perf_mode=DR,
                        )
                        nc.tensor.matmul(
                            d_ps[:, :w],
                            ones8[:],
                            et_cur[:, 2 * p_i : 2 * p_i + 2, :],
                            start=first,
                            stop=False,
                            perf_mode=DR,
                        )
                else:
                    nc.tensor.matmul(
                        y_ps[:, :w],
                        gT[:, (MB - 1) * P : MB * P],
                        et_cur[:, :w],
                        start=False,
                        stop=True,
                    )
                    nc.tensor.matmul(
                        d_ps[:, :w],
                        ones8[:, 0, :],
                        et_cur[:, :w],
                        start=False,
                        stop=True,
                    )
            return y_ps, d_ps

        def make_epilogue(ci, cs, w, y_ps, d_ps):
            def _ep():
                # normalize: ynorm = y / d  (d replicated across partitions)
                r_t = rp.tile([P, CW], F32, tag="r", name="r")
                nc.vector.reciprocal_approx_fast(
                    out=r_t[:, :w], in_=d_ps[:, :w]
                )
                nc.vector.tensor_mul(
                    out=ynorm[:, cs : cs + w], in0=y_ps[:, :w], in1=r_t[:, :w]
                )
                # z = w_z @ ynorm + x -> out. h=0 reuses the d slot (freed
                # after the recip), h=1 the y slot.
                if ci == len(CHUNKS) - 1:
                    # final epilogue: fill the PE's recip/mul wait with
                    # dummies so the closing z matmuls run at full clock
                    fil2 = psum.tile([P, CW], F32, tag="s", name="fil2")
                    for _ in range(10):
                        nc.tensor.matmul(
                            fil2[:], warm[:, :P], warm[:], start=True, stop=True
                        )
                for h, ztag in ((0, "d"), (1, "y")):
                    zpool = psum_y if ztag == "y" else psum_d
                    z_ps = zpool.tile([P, CW], F32, tag=ztag, name="zps")
                    nc.tensor.matmul(
                        z_ps[:, :w],
                        wzT[h][:],
                        ynorm[:, cs : cs + w],
                        start=True,
                        stop=True,
                    )
                    o_t = outp.tile([P, CW], F32, tag="o", name="o")
                    nc.vector.tensor_add(
                        out=o_t[:, :w], in0=z_ps[:, :w], in1=x_f32[h][:, cs : cs + w]
                    )
                    half = (w + 1) // 2
                    final = ci == len(CHUNKS) - 1
                    for s, (so, sw) in enumerate(((0, half), (half, w - half))):
                        if sw <= 0:
                            continue
                        if final:
                            # last chunk: fan the stores over 3 queues so the
                            # closing store flight is as short as possible
                            eng = (nc.sync, nc.gpsimd, nc.scalar, nc.sync)[
                                2 * h + s
                            ]
                        else:
                            eng = out_dma_engs[(h + ci + s) % 2]
                        eng.dma_start(
                            out=out_d[h * P : (h + 1) * P, cs + so : cs + so + sw],
                            in_=o_t[:, so : so + sw],
                        )

            return _ep

        pending = [None]

        def flush():
            if pending[0] is not None:
                pending[0]()
                pending[0] = None

        # Steady chunks split every pair-unit's exp across ACT+DVE (see
        # emit_scores_exp). Chunk 0's DVE is busy with interleaved
        # projections -> ACT-only there; the tail m-unit stays on DVE.
        DVE_UNITS = (1, 4, 7, 10, 12)

        def steady_units(u):
            return u in DVE_UNITS

        for ci, (cs, w) in enumerate(CHUNKS):
            if w > 64:
                y_ps, d_ps = wide_chunk(
                    cs,
                    w,
                    interleave=(ci == 0),
                    flush=flush,
                    dve_mode=(lambda u: False) if ci == 0 else steady_units,
                )
            else:
                y_ps, d_ps = tail_chunk(cs, w, flush=flush)
            pending[0] = make_epilogue(ci, cs, w, y_ps, d_ps)
        flush()

        assert done == {"th": NP_CH, "ph": NP_CH, "gT": MB}, done


_NC_CACHE = None


def build_nc():
    global _NC_CACHE
    if _NC_CACHE is None:
        nc = bacc.Bacc("TRN2", target_bir_lowering=False, debug=False)
        with tile.TileContext(nc) as tc:
            _kernel_body(tc)
        nc.compile()
        _NC_CACHE = nc
    return _NC_CACHE


def kernel(x, w_theta, w_phi, w_g, w_z, trace=False):
    assert x.shape == (B, C, H, W), x.shape
    nc = build_nc()
    from concourse.bass_utils import run_bass_kernel_spmd

    # pack all weights into one [128, 1024] f32 tensor (see _kernel_body):
    # [ th_j0 | ph_j0 | g_j0 | th_j1 | ph_j1 | g_j1 | wz.T ] with
    # w_k_jh[p, m] = w_k[m, h*128+p]
    wth_t = np.asarray(w_theta, dtype=np.float32).T  # [C, CH]
    wph_t = np.asarray(w_phi, dtype=np.float32).T
    wg_t = np.asarray(w_g, dtype=np.float32).T
    wz_t = np.asarray(w_z, dtype=np.float32).T  # [CH, C]
    blocks = []
    for h in range(2):
        for wt in (wth_t, wph_t, wg_t):
            blocks.append(wt[h * 128 : (h + 1) * 128, :])
    blocks.append(wz_t)
    shared = {
        "w_cat": np.ascontiguousarray(np.concatenate(blocks, axis=1), np.float32)
    }
    in_maps = [
        dict(shared, x=np.ascontiguousarray(x[i].reshape(C, N), dtype=np.float32))
        for i in range(N_CORES)
    ]
    res = run_bass_kernel_spmd(
        nc, in_maps, core_ids=list(range(N_CORES)), trace=trace
    )
    out = np.stack([res.results[i]["out"].reshape(C, H, W) for i in range(N_CORES)])
    kernel.last_result = res
    return out


kernel.last_result = None

